# revision 19
# baseline (speedup 1.0000x reference)
"""Trainium2 Bass kernel for nn_Attention_4183298146960 — v6 sharding.

GQA causal attention layer: B=2, S=2048, HIDDEN=2048, 16 q heads / 4 kv heads,
head_dim=128, RoPE (interleaved pairs), causal softmax, output projection.

Sharding v6 (8 cores, SPMD-uniform program, per-core inputs differ):
  core c owns batch c//4 and kv-head g=c%4 with its 4 q heads {4g..4g+3}.
  Eliminates the duplicated K/V projections of head-sharding (each (batch,
  kv-head) is computed exactly once), halves the x DMA per core, and the
  output-projection gather becomes an AllGather within each 4-core batch
  group (6 MB received per core instead of 14 MB).

Pipeline: per 512-token tile j (= qt chunk of my batch): QKV (q0..q3,k,v as
six N=512 column blocks of one packed weight), RoPE on q/k, PE-transpose of
v to token-major, then the causal attention chunk over k-tiles 0..4qt+3 in
two head-pair passes, its AllGather, the previous chunk's gather-readback
(emitted before the new AllGather so the framework's shared collective
counter waits only on the producing gather), and a deferred chunk's W_o.

Layouts as before: feature-major qT/kT for scores, token-major v for PV,
scores [k, q] with a head-pair side by side in one [128, 1024] PSUM tile;
unnormalized exp, ones-matmul denominator, late division.
"""

import numpy as np
import ml_dtypes

import concourse.bass as bass
import concourse.mybir as mybir
import concourse.tile as tile
from concourse import bacc
from concourse.bass_utils import run_bass_kernel_spmd

BF16 = ml_dtypes.bfloat16

HEADS = 16
KV_HEADS = 4
HIDDEN = 2048
HD = 128
S = 2048
B = 2
HT = HIDDEN // 128             # 16 hidden tiles
NQ = 4                         # local q heads
SCALE = 1.0 / float(np.sqrt(HD))
RG8 = [[0, 1, 2, 3, 4, 5, 6, 7]]

_COMPILED = None


def _build():
    dt = mybir.dt
    nc = bacc.Bacc("TRN2", target_bir_lowering=False, debug=False, num_devices=8)

    xT = nc.dram_tensor("xT", [128, HT, S], dt.bfloat16, kind="ExternalInput")
    wqkv = nc.dram_tensor("wqkv", [128, HT, 768], dt.bfloat16, kind="ExternalInput")
    wo = nc.dram_tensor("wo", [128, HT, 256], dt.bfloat16, kind="ExternalInput")
    cc = nc.dram_tensor("cc", [128, S], dt.bfloat16, kind="ExternalInput")
    ss = nc.dram_tensor("ss", [128, S], dt.bfloat16, kind="ExternalInput")
    msk = nc.dram_tensor("msk", [128, 4, 1024], dt.bfloat16, kind="ExternalInput")
    ones128 = nc.dram_tensor("ones128", [128, 128], dt.bfloat16, kind="ExternalInput")
    ident = nc.dram_tensor("ident", [128, 128], dt.bfloat16, kind="ExternalInput")
    outT = nc.dram_tensor("outT", [256, B * S], dt.float32, kind="ExternalOutput")

    mult = mybir.AluOpType.mult
    add = mybir.AluOpType.add
    Exp = mybir.ActivationFunctionType.Exp

    with tile.TileContext(nc) as tc:
        with (
            tc.tile_pool(name="const", bufs=1) as constp,
            tc.tile_pool(name="dram", bufs=1, space="DRAM") as dram,
            tc.tile_pool(name="xp", bufs=2) as xp,
            tc.tile_pool(name="rp", bufs=3) as rp,
            tc.tile_pool(name="probs", bufs=4) as probs,
            tc.tile_pool(name="smallp", bufs=2) as smallp,
            tc.tile_pool(name="ap", bufs=3) as apool,
            tc.tile_pool(name="wosb", bufs=3) as wosb,
            tc.tile_pool(name="outp", bufs=2) as outp,
            # PSUM: qk 2 banks + scores 2x2 banks + pv 2 banks = 8
            tc.tile_pool(name="qkps", bufs=2, space="PSUM") as qkps,
            tc.tile_pool(name="spool", bufs=2, space="PSUM") as spool,
            tc.tile_pool(name="pvp", bufs=1, space="PSUM") as pvp,
        ):
            qcat = constp.tile([128, NQ * S], dt.bfloat16)   # 4 local q heads
            kT = constp.tile([128, S], dt.bfloat16)
            vsb = constp.tile([128, S], dt.bfloat16)         # token-major v
            wqkv_sb = constp.tile([128, HT, 768], dt.bfloat16)
            wo_sb = constp.tile([128, HT, 256], dt.bfloat16)
            cc_sb = constp.tile([128, S], dt.bfloat16)
            ss_sb = constp.tile([128, S], dt.bfloat16)
            msk_sb = constp.tile([128, 4, 1024], dt.bfloat16)
            ones_sb = constp.tile([128, 128], dt.bfloat16)
            id_sb = constp.tile([128, 128], dt.bfloat16)

            def load_x(j, x_sb):
                for hq in range(4):
                    nc.sync.dma_start(
                        x_sb[:, hq * 4:(hq + 1) * 4, :],
                        xT[:, hq * 4:(hq + 1) * 4, j * 512:(j + 1) * 512],
                    )

            # interleave weight/x chunks so the first ht-tile MMs start ASAP
            x_tiles = {}
            x_tiles[0] = xp.tile([128, HT, 512], dt.bfloat16, name="x0", tag="x")
            for lo, hi in ((0, 2), (2, 4), (4, 8), (8, 12), (12, 16)):
                nc.sync.dma_start(wqkv_sb[:, lo:hi, :], wqkv[:, lo:hi, :])
                nc.sync.dma_start(
                    x_tiles[0][:, lo:hi, :], xT[:, lo:hi, 0:512],
                )

            nc.sync.dma_start(cc_sb[:, 0:512], cc[:, 0:512])
            nc.sync.dma_start(ss_sb[:, 0:512], ss[:, 0:512])
            nc.sync.dma_start(id_sb[:], ident[:])
            nc.sync.dma_start(cc_sb[:, 512:S], cc[:, 512:S])
            nc.sync.dma_start(ss_sb[:, 512:S], ss[:, 512:S])
            nc.sync.dma_start(msk_sb[:], msk[:])
            nc.sync.dma_start(ones_sb[:], ones128[:])
            nc.sync.dma_start(wo_sb[:], wo[:])

            def emit_qkv(j, x_sb):
                """QKV + RoPE + v-transpose for t-tile j (512 tokens)."""
                tsl = bass.ts(j, 512)
                for ft in range(6):
                    ps = qkps.tile([128, 512], dt.float32, tag="qk")
                    for ht in range(HT):
                        nc.tensor.matmul(
                            ps[:],
                            lhsT=wqkv_sb[:, ht, ft * 128:(ft + 1) * 128],
                            rhs=x_sb[:, ht, :],
                            start=(ht == 0),
                            stop=(ht == HT - 1),
                        )
                    if ft < 5:
                        # q0..q3, k: RoPE
                        sbq = rp.tile([128, 512], dt.bfloat16, name="sbq")
                        nc.scalar.copy(sbq[:], ps[:])
                        tmp = rp.tile([128, 512], dt.bfloat16, name="tmp")
                        nc.gpsimd.dma_start(tmp[0:64, :], sbq[64:128, :])
                        nc.gpsimd.dma_start(tmp[64:128, :], sbq[0:64, :])
                        qcc = rp.tile([128, 512], dt.bfloat16, name="qcc")
                        nc.vector.tensor_tensor(qcc[:], sbq[:], cc_sb[:, tsl], mult)
                        qss = rp.tile([128, 512], dt.bfloat16, name="qss")
                        nc.vector.tensor_tensor(qss[:], tmp[:], ss_sb[:, tsl], mult)
                        if ft < 4:
                            dst = qcat[:, ft * S + j * 512: ft * S + (j + 1) * 512]
                        else:
                            dst = kT[:, tsl]
                        nc.vector.tensor_tensor(dst, qcc[:], qss[:], add)
                    else:
                        # v: feature-major -> PE transpose -> token-major
                        vf = rp.tile([128, 512], dt.bfloat16, name="vf")
                        nc.scalar.copy(vf[:], ps[:])
                        pt = qkps.tile(
                            [128, 512], dt.bfloat16, tag="qk", name="pt",
                            padded_shape=[128, 1024],
                        )
                        for st in range(4):
                            nc.tensor.transpose(
                                pt[:, st * 128:(st + 1) * 128],
                                vf[:, st * 128:(st + 1) * 128],
                                id_sb[:],
                            )
                        nc.vector.tensor_copy(vsb[:, tsl], pt[:])

            def emit_attn_pass(qt, hp):
                """Causal attention for 512 q tokens, one head pair; ends with
                the pair's 8-rank AllGather (mesh algorithm; the gathered
                tensor holds both batch groups — W_o reads are batch-split by
                output token range, so the program stays SPMD-uniform)."""
                kts = 4 * qt + 4
                attn_chunk = dram.tile(
                    [256, 512], dt.bfloat16, name=f"attnc{qt}_{hp}", tag=f"ac{qt}{hp}"
                )
                pv = pvp.tile([128, 1024], dt.float32, tag="pv")
                acc = smallp.tile([128, 1024], dt.bfloat16, name="acc", tag="acc")
                for kt in range(kts):
                    r = kt - 4 * qt
                    ps_s = spool.tile([128, 1024], dt.float32, tag="sc")
                    for hl in range(2):
                        h = 2 * hp + hl
                        nc.tensor.matmul(
                            ps_s[:, hl * 512:(hl + 1) * 512],
                            lhsT=kT[:, kt * 128:(kt + 1) * 128],
                            rhs=qcat[:, h * S + qt * 512: h * S + (qt + 1) * 512],
                            start=True,
                            stop=True,
                        )
                    prob = probs.tile([128, 1024], dt.bfloat16, tag="prob")
                    nc.scalar.activation(prob[:], ps_s[:], Exp, scale=SCALE)
                    if r >= 0:
                        nc.vector.tensor_tensor(
                            prob[:], prob[:], msk_sb[:, r, :], mult
                        )
                    for hl in range(2):
                        nc.tensor.matmul(
                            pv[:, hl * 512:(hl + 1) * 512],
                            lhsT=vsb[:, kt * 128:(kt + 1) * 128],
                            rhs=prob[:, hl * 512:(hl + 1) * 512],
                            start=(kt == 0), stop=(kt == kts - 1),
                        )
                    if kt == 0:
                        nc.vector.tensor_copy(acc[:], prob[:])
                    else:
                        nc.vector.tensor_add(acc[:], acc[:], prob[:])
                ps_den = spool.tile([128, 1024], dt.float32, tag="sc", name="ps_den")
                for hl in range(2):
                    nc.tensor.matmul(
                        ps_den[:, hl * 512:(hl + 1) * 512],
                        lhsT=ones_sb[:], rhs=acc[:, hl * 512:(hl + 1) * 512],
                        start=True, stop=True,
                    )
                den_sb = smallp.tile([128, 1024], dt.float32, name="den", tag="den")
                nc.vector.reciprocal_approx_fast(den_sb[:], ps_den[:])
                for hl in range(2):
                    attn_sb = apool.tile([128, 512], dt.bfloat16, tag="asb")
                    nc.vector.tensor_tensor(
                        attn_sb[:], pv[:, hl * 512:(hl + 1) * 512],
                        den_sb[:, hl * 512:(hl + 1) * 512], mult,
                    )
                    nc.sync.dma_start(
                        attn_chunk[hl * 128:(hl + 1) * 128, :], attn_sb[:]
                    )
                ag_out = dram.tile(
                    [HT, 128, 512], dt.bfloat16,
                    addr_space="Shared", name=f"agout{qt}_{hp}", tag=f"ag{qt}{hp}",
                )
                nc.gpsimd.collective_compute(
                    "AllGather", mybir.AluOpType.bypass, replica_groups=RG8,
                    ins=[attn_chunk.opt()], outs=[ag_out.opt()],
                )
                return ag_out

            def load_asb(ag_out):
                # NB: must be emitted BEFORE the next collective so the
                # framework's shared collective-completion counter makes this
                # wait only on the producing AllGather, not a later one.
                asb = wosb.tile([128, HT, 512], dt.bfloat16, tag="asb")
                for g in range(4):
                    nc.sync.dma_start(
                        asb[:, g * 4:(g + 1) * 4, :],
                        ag_out[g * 4:(g + 1) * 4, :, :].transpose([1, 0, 2]),
                    )
                return asb

            def emit_wo(qt, asb01):
                """W_o for one chunk: my 256 output columns for both batches'
                512 tokens, contracting over both head-pair gathers.
                Gathered rows: blocks 0-7 batch0, 8-15 batch1 (rank-major)."""
                for b in range(B):
                    for ct in range(2):
                        ps_o = qkps.tile([128, 512], dt.float32, tag="qk", name="ps_o")
                        for hp in range(2):
                            for t in range(8):
                                nc.tensor.matmul(
                                    ps_o[:],
                                    lhsT=wo_sb[:, hp * 8 + t, ct * 128:(ct + 1) * 128],
                                    rhs=asb01[hp][:, b * 8 + t, :],
                                    start=(hp == 0 and t == 0),
                                    stop=(hp == 1 and t == 7),
                                )
                        o_sb = outp.tile([128, 512], dt.float32, tag="osb")
                        nc.scalar.copy(o_sb[:], ps_o[:])
                        nc.scalar.dma_start(
                            outT[ct * 128:(ct + 1) * 128,
                                 b * S + qt * 512:b * S + (qt + 1) * 512],
                            o_sb[:],
                        )

            pending_ag = []    # gathered passes, asb not yet loaded
            loaded = []        # (qt, [asb_hp0, asb_hp1]) ready for W_o
            for j in range(4):
                if j + 1 < 4:
                    x_tiles[j + 1] = xp.tile(
                        [128, HT, 512], dt.bfloat16, name=f"x{j + 1}", tag="x"
                    )
                    load_x(j + 1, x_tiles[j + 1])
                emit_qkv(j, x_tiles[j])
                # read back the previous chunk's two gathers before emitting
                # this iteration's AllGathers
                if pending_ag:
                    qq, ag0, ag1 = pending_ag.pop(0)
                    loaded.append((qq, [load_asb(ag0), load_asb(ag1)]))
                ag0 = emit_attn_pass(j, 0)
                if loaded:
                    qq, asb01 = loaded.pop(0)
                    emit_wo(qq, asb01)
                ag1 = emit_attn_pass(j, 1)
                pending_ag.append((j, ag0, ag1))
            # tail: chunk 3's gathers
            qq, ag0, ag1 = pending_ag.pop(0)
            asb01 = [load_asb(ag0), load_asb(ag1)]
            emit_wo(qq, asb01)
    nc.compile()
    return nc


# host-side input prep ------------------------------------------------------

_PERM = np.concatenate([np.arange(0, HD, 2), np.arange(1, HD, 2)])


def _rope_tables():
    freq = 1.0 / (10000.0 ** (np.arange(0, HD, 2, dtype=np.float64) / HD))
    pos = np.arange(S, dtype=np.float64)
    ang = np.outer(pos, freq)                       # [S, 64]
    cos = np.cos(ang).T.astype(np.float32)          # [64, S]
    sin = np.sin(ang).T.astype(np.float32)
    cc1 = np.concatenate([cos, cos], 0)             # [128, S]
    ss1 = np.concatenate([-sin, sin], 0)            # [128, S]
    return cc1.astype(BF16), ss1.astype(BF16)


def _prep_inputs(x, W_qkv, W_o):
    x = np.asarray(x, dtype=np.float32)
    W_qkv = np.asarray(W_qkv, dtype=np.float32)
    W_o = np.asarray(W_o, dtype=np.float32)

    xTd = [
        np.ascontiguousarray(
            x[b].T.reshape(HT, 128, S).transpose(1, 0, 2)
        ).astype(BF16)                               # [128, HT, 2048]
        for b in range(B)
    ]

    cc, ss = _rope_tables()

    mask = np.zeros((128, 4, 1024), dtype=np.float32)
    ii = np.arange(128)[:, None]
    jj = np.arange(512)[None, :]
    for r in range(4):
        m = (jj >= ii + 128 * r)
        mask[:, r, 0:512] = m
        mask[:, r, 512:1024] = m
    mask = mask.astype(BF16)

    ones128 = np.ones((128, 128), dtype=np.float32).astype(BF16)
    ident = np.eye(128, dtype=np.float32).astype(BF16)

    # gathered feature-tile order: block hp*8+t holds feature tile
    # f128 = 4*(t//2) + 2*hp + t%2  (rank-major AllGather of head pairs)
    _DTT = [4 * (t // 2) + 2 * hp + t % 2 for hp in range(2) for t in range(8)]

    in_maps = []
    for c in range(8):
        b, g = c // 4, c % 4
        qr = W_qkv[512 * g: 512 * (g + 1)]           # rows of q heads 4g..4g+3
        qr = qr.reshape(NQ, HD, HIDDEN)[:, _PERM, :].reshape(512, HIDDEN)
        kr = W_qkv[HIDDEN + 128 * g: HIDDEN + 128 * (g + 1)][_PERM, :]
        vr = W_qkv[HIDDEN + 512 + 128 * g: HIDDEN + 512 + 128 * (g + 1)]
        wqkvT = np.ascontiguousarray(
            np.concatenate([qr, kr, vr], 0).T.reshape(HT, 128, 768).transpose(1, 0, 2)
        ).astype(BF16)                               # [128, HT, 768]
        woT = np.ascontiguousarray(
            W_o[256 * c: 256 * (c + 1)].T.reshape(HT, 128, 256)[_DTT].transpose(1, 0, 2)
        ).astype(BF16)                               # [128, HT(reordered), 256]
        in_maps.append({
            "xT": xTd[b], "wqkv": wqkvT, "wo": woT,
            "cc": cc, "ss": ss, "msk": mask, "ones128": ones128, "ident": ident,
        })
    return in_maps


def kernel(x, W_qkv, W_o):
    global _COMPILED
    if _COMPILED is None:
        _COMPILED = _build()
    nc = _COMPILED
    in_maps = _prep_inputs(x, W_qkv, W_o)
    res = run_bass_kernel_spmd(nc, in_maps, list(range(8)))
    out = np.empty((B, S, HIDDEN), dtype=np.float32)
    for c in range(8):
        oT = res.results[c]["outT"]                  # [256, 4096]
        out[:, :, 256 * c: 256 * (c + 1)] = oT.reshape(256, B, S).transpose(1, 2, 0)
    return out


# revision 20
# speedup vs baseline: 1.0258x; 1.0258x over previous
"""Trainium2 Bass kernel for nn_Attention_4183298146960 — v6 sharding.

GQA causal attention layer: B=2, S=2048, HIDDEN=2048, 16 q heads / 4 kv heads,
head_dim=128, RoPE (interleaved pairs), causal softmax, output projection.

Sharding v6 (8 cores, SPMD-uniform program, per-core inputs differ):
  core c owns batch c//4 and kv-head g=c%4 with its 4 q heads {4g..4g+3}.
  Eliminates the duplicated K/V projections of head-sharding (each (batch,
  kv-head) is computed exactly once), halves the x DMA per core, and the
  output-projection gather becomes an AllGather within each 4-core batch
  group (6 MB received per core instead of 14 MB).

Pipeline: per 512-token tile j (= qt chunk of my batch): QKV (q0..q3,k,v as
six N=512 column blocks of one packed weight), RoPE on q/k, PE-transpose of
v to token-major, then the causal attention chunk over k-tiles 0..4qt+3 in
two head-pair passes, its AllGather, the previous chunk's gather-readback
(emitted before the new AllGather so the framework's shared collective
counter waits only on the producing gather), and a deferred chunk's W_o.

Layouts as before: feature-major qT/kT for scores, token-major v for PV,
scores [k, q] with a head-pair side by side in one [128, 1024] PSUM tile;
unnormalized exp, ones-matmul denominator, late division.
"""

import numpy as np
import ml_dtypes

import concourse.bass as bass
import concourse.mybir as mybir
import concourse.tile as tile
from concourse import bacc
from concourse.bass_utils import run_bass_kernel_spmd

BF16 = ml_dtypes.bfloat16

HEADS = 16
KV_HEADS = 4
HIDDEN = 2048
HD = 128
S = 2048
B = 2
HT = HIDDEN // 128             # 16 hidden tiles
NQ = 4                         # local q heads
SCALE = 1.0 / float(np.sqrt(HD))
RG8 = [[0, 1, 2, 3, 4, 5, 6, 7]]

_COMPILED = None


def _build():
    dt = mybir.dt
    nc = bacc.Bacc("TRN2", target_bir_lowering=False, debug=False, num_devices=8)

    xT = nc.dram_tensor("xT", [128, HT, S], dt.bfloat16, kind="ExternalInput")
    wqkv = nc.dram_tensor("wqkv", [128, HT, 768], dt.bfloat16, kind="ExternalInput")
    wo = nc.dram_tensor("wo", [128, HT, 256], dt.bfloat16, kind="ExternalInput")
    cc = nc.dram_tensor("cc", [128, S], dt.bfloat16, kind="ExternalInput")
    ss = nc.dram_tensor("ss", [128, S], dt.bfloat16, kind="ExternalInput")
    msk = nc.dram_tensor("msk", [128, 4, 1024], dt.bfloat16, kind="ExternalInput")
    ones128 = nc.dram_tensor("ones128", [128, 128], dt.bfloat16, kind="ExternalInput")
    ident = nc.dram_tensor("ident", [128, 128], dt.bfloat16, kind="ExternalInput")
    outT = nc.dram_tensor("outT", [256, B * S], dt.float32, kind="ExternalOutput")

    mult = mybir.AluOpType.mult
    add = mybir.AluOpType.add
    Exp = mybir.ActivationFunctionType.Exp

    with tile.TileContext(nc) as tc:
        with (
            tc.tile_pool(name="const", bufs=1) as constp,
            tc.tile_pool(name="dram", bufs=1, space="DRAM") as dram,
            tc.tile_pool(name="xp", bufs=2) as xp,
            tc.tile_pool(name="rp", bufs=3) as rp,
            tc.tile_pool(name="probs", bufs=3) as probs,
            tc.tile_pool(name="smallp", bufs=2) as smallp,
            tc.tile_pool(name="ap", bufs=3) as apool,
            tc.tile_pool(name="wosb", bufs=4) as wosb,
            tc.tile_pool(name="outp", bufs=1) as outp,
            # PSUM: qk 2 banks + scores 2x2 banks + pv 2 banks = 8
            tc.tile_pool(name="qkps", bufs=2, space="PSUM") as qkps,
            tc.tile_pool(name="spool", bufs=2, space="PSUM") as spool,
            tc.tile_pool(name="pvp", bufs=1, space="PSUM") as pvp,
        ):
            qcat = constp.tile([128, NQ * S], dt.bfloat16)   # 4 local q heads
            kT = constp.tile([128, S], dt.bfloat16)
            vsb = constp.tile([128, S], dt.bfloat16)         # token-major v
            wqkv_sb = constp.tile([128, HT, 768], dt.bfloat16)
            wo_sb = constp.tile([128, HT, 256], dt.bfloat16)
            cc_sb = constp.tile([128, S], dt.bfloat16)
            ss_sb = constp.tile([128, S], dt.bfloat16)
            msk_sb = constp.tile([128, 4, 1024], dt.bfloat16)
            ones_sb = constp.tile([128, 128], dt.bfloat16)
            id_sb = constp.tile([128, 128], dt.bfloat16)

            def load_x(j, x_sb):
                for hq in range(4):
                    nc.sync.dma_start(
                        x_sb[:, hq * 4:(hq + 1) * 4, :],
                        xT[:, hq * 4:(hq + 1) * 4, j * 512:(j + 1) * 512],
                    )

            # interleave weight/x chunks so the first ht-tile MMs start ASAP
            x_tiles = {}
            x_tiles[0] = xp.tile([128, HT, 512], dt.bfloat16, name="x0", tag="x")
            for lo, hi in ((0, 2), (2, 4), (4, 8), (8, 12), (12, 16)):
                nc.sync.dma_start(wqkv_sb[:, lo:hi, :], wqkv[:, lo:hi, :])
                nc.sync.dma_start(
                    x_tiles[0][:, lo:hi, :], xT[:, lo:hi, 0:512],
                )

            nc.sync.dma_start(cc_sb[:, 0:512], cc[:, 0:512])
            nc.sync.dma_start(ss_sb[:, 0:512], ss[:, 0:512])
            nc.sync.dma_start(id_sb[:], ident[:])
            nc.sync.dma_start(cc_sb[:, 512:S], cc[:, 512:S])
            nc.sync.dma_start(ss_sb[:, 512:S], ss[:, 512:S])
            nc.sync.dma_start(msk_sb[:], msk[:])
            nc.sync.dma_start(ones_sb[:], ones128[:])
            nc.sync.dma_start(wo_sb[:], wo[:])

            def emit_qkv(j, x_sb):
                """QKV + RoPE + v-transpose for t-tile j (512 tokens)."""
                tsl = bass.ts(j, 512)
                for ft in range(6):
                    ps = qkps.tile([128, 512], dt.float32, tag="qk")
                    for ht in range(HT):
                        nc.tensor.matmul(
                            ps[:],
                            lhsT=wqkv_sb[:, ht, ft * 128:(ft + 1) * 128],
                            rhs=x_sb[:, ht, :],
                            start=(ht == 0),
                            stop=(ht == HT - 1),
                        )
                    if ft < 5:
                        # q0..q3, k: RoPE
                        sbq = rp.tile([128, 512], dt.bfloat16, name="sbq")
                        nc.scalar.copy(sbq[:], ps[:])
                        tmp = rp.tile([128, 512], dt.bfloat16, name="tmp")
                        nc.gpsimd.dma_start(tmp[0:64, :], sbq[64:128, :])
                        nc.gpsimd.dma_start(tmp[64:128, :], sbq[0:64, :])
                        qcc = rp.tile([128, 512], dt.bfloat16, name="qcc")
                        nc.vector.tensor_tensor(qcc[:], sbq[:], cc_sb[:, tsl], mult)
                        qss = rp.tile([128, 512], dt.bfloat16, name="qss")
                        nc.vector.tensor_tensor(qss[:], tmp[:], ss_sb[:, tsl], mult)
                        if ft < 4:
                            dst = qcat[:, ft * S + j * 512: ft * S + (j + 1) * 512]
                        else:
                            dst = kT[:, tsl]
                        nc.vector.tensor_tensor(dst, qcc[:], qss[:], add)
                    else:
                        # v: feature-major -> PE transpose -> token-major
                        vf = rp.tile([128, 512], dt.bfloat16, name="vf")
                        nc.scalar.copy(vf[:], ps[:])
                        pt = qkps.tile(
                            [128, 512], dt.bfloat16, tag="qk", name="pt",
                            padded_shape=[128, 1024],
                        )
                        for st in range(4):
                            nc.tensor.transpose(
                                pt[:, st * 128:(st + 1) * 128],
                                vf[:, st * 128:(st + 1) * 128],
                                id_sb[:],
                            )
                        nc.vector.tensor_copy(vsb[:, tsl], pt[:])

            def emit_attn_pass(qt, hp):
                """Causal attention for 512 q tokens, one head pair; ends with
                the pair's 8-rank AllGather (mesh algorithm; the gathered
                tensor holds both batch groups — W_o reads are batch-split by
                output token range, so the program stays SPMD-uniform)."""
                kts = 4 * qt + 4
                attn_chunk = dram.tile(
                    [256, 512], dt.bfloat16, name=f"attnc{qt}_{hp}", tag=f"ac{qt}{hp}"
                )
                pv = pvp.tile([128, 1024], dt.float32, tag="pv")
                acc = smallp.tile([128, 1024], dt.bfloat16, name="acc", tag="acc")
                for kt in range(kts):
                    r = kt - 4 * qt
                    ps_s = spool.tile([128, 1024], dt.float32, tag="sc")
                    for hl in range(2):
                        h = 2 * hp + hl
                        nc.tensor.matmul(
                            ps_s[:, hl * 512:(hl + 1) * 512],
                            lhsT=kT[:, kt * 128:(kt + 1) * 128],
                            rhs=qcat[:, h * S + qt * 512: h * S + (qt + 1) * 512],
                            start=True,
                            stop=True,
                        )
                    prob = probs.tile([128, 1024], dt.bfloat16, tag="prob")
                    nc.scalar.activation(prob[:], ps_s[:], Exp, scale=SCALE)
                    if r >= 0:
                        nc.vector.tensor_tensor(
                            prob[:], prob[:], msk_sb[:, r, :], mult
                        )
                    for hl in range(2):
                        nc.tensor.matmul(
                            pv[:, hl * 512:(hl + 1) * 512],
                            lhsT=vsb[:, kt * 128:(kt + 1) * 128],
                            rhs=prob[:, hl * 512:(hl + 1) * 512],
                            start=(kt == 0), stop=(kt == kts - 1),
                        )
                    if kt == 0:
                        nc.vector.tensor_copy(acc[:], prob[:])
                    else:
                        nc.vector.tensor_add(acc[:], acc[:], prob[:])
                ps_den = spool.tile([128, 1024], dt.float32, tag="sc", name="ps_den")
                for hl in range(2):
                    nc.tensor.matmul(
                        ps_den[:, hl * 512:(hl + 1) * 512],
                        lhsT=ones_sb[:], rhs=acc[:, hl * 512:(hl + 1) * 512],
                        start=True, stop=True,
                    )
                den_sb = smallp.tile([128, 1024], dt.float32, name="den", tag="den")
                nc.vector.reciprocal_approx_fast(den_sb[:], ps_den[:])
                for hl in range(2):
                    attn_sb = apool.tile([128, 512], dt.bfloat16, tag="asb")
                    nc.vector.tensor_tensor(
                        attn_sb[:], pv[:, hl * 512:(hl + 1) * 512],
                        den_sb[:, hl * 512:(hl + 1) * 512], mult,
                    )
                    nc.sync.dma_start(
                        attn_chunk[hl * 128:(hl + 1) * 128, :], attn_sb[:]
                    )
                ag_out = dram.tile(
                    [HT, 128, 512], dt.bfloat16,
                    addr_space="Shared", name=f"agout{qt}_{hp}", tag=f"ag{qt}{hp}",
                )
                nc.gpsimd.collective_compute(
                    "AllGather", mybir.AluOpType.bypass, replica_groups=RG8,
                    ins=[attn_chunk.opt()], outs=[ag_out.opt()],
                )
                return ag_out

            def load_asb(ag_out):
                # NB: must be emitted BEFORE the next collective so the
                # framework's shared collective-completion counter makes this
                # wait only on the producing AllGather, not a later one.
                asb = wosb.tile([128, HT, 512], dt.bfloat16, tag="asb")
                for g in range(4):
                    nc.sync.dma_start(
                        asb[:, g * 4:(g + 1) * 4, :],
                        ag_out[g * 4:(g + 1) * 4, :, :].transpose([1, 0, 2]),
                    )
                return asb

            def emit_wo(qt, asb01):
                """W_o for one chunk: my 256 output columns for both batches'
                512 tokens, contracting over both head-pair gathers.
                Gathered rows: blocks 0-7 batch0, 8-15 batch1 (rank-major)."""
                for b in range(B):
                    for ct in range(2):
                        ps_o = qkps.tile([128, 512], dt.float32, tag="qk", name="ps_o")
                        for hp in range(2):
                            for t in range(8):
                                nc.tensor.matmul(
                                    ps_o[:],
                                    lhsT=wo_sb[:, hp * 8 + t, ct * 128:(ct + 1) * 128],
                                    rhs=asb01[hp][:, b * 8 + t, :],
                                    start=(hp == 0 and t == 0),
                                    stop=(hp == 1 and t == 7),
                                )
                        o_sb = outp.tile([128, 512], dt.float32, tag="osb")
                        nc.scalar.copy(o_sb[:], ps_o[:])
                        nc.scalar.dma_start(
                            outT[ct * 128:(ct + 1) * 128,
                                 b * S + qt * 512:b * S + (qt + 1) * 512],
                            o_sb[:],
                        )

            pending_ag = []    # gathered passes, asb not yet loaded
            loaded = []        # (qt, [asb_hp0, asb_hp1]) ready for W_o
            for j in range(4):
                if j + 1 < 4:
                    x_tiles[j + 1] = xp.tile(
                        [128, HT, 512], dt.bfloat16, name=f"x{j + 1}", tag="x"
                    )
                    load_x(j + 1, x_tiles[j + 1])
                emit_qkv(j, x_tiles[j])
                # read back the previous chunk's two gathers before emitting
                # this iteration's AllGathers
                if pending_ag:
                    qq, ag0, ag1 = pending_ag.pop(0)
                    loaded.append((qq, [load_asb(ag0), load_asb(ag1)]))
                ag0 = emit_attn_pass(j, 0)
                if len(loaded) > 1:
                    qq, asb01 = loaded.pop(0)
                    emit_wo(qq, asb01)
                ag1 = emit_attn_pass(j, 1)
                pending_ag.append((j, ag0, ag1))
            # tail: chunk 2 (already loaded) covers the final gather's
            # latency, then chunk 3
            qq, asb01 = loaded.pop(0)
            emit_wo(qq, asb01)
            qq, ag0, ag1 = pending_ag.pop(0)
            asb01 = [load_asb(ag0), load_asb(ag1)]
            emit_wo(qq, asb01)
    nc.compile()
    return nc


# host-side input prep ------------------------------------------------------

_PERM = np.concatenate([np.arange(0, HD, 2), np.arange(1, HD, 2)])


def _rope_tables():
    freq = 1.0 / (10000.0 ** (np.arange(0, HD, 2, dtype=np.float64) / HD))
    pos = np.arange(S, dtype=np.float64)
    ang = np.outer(pos, freq)                       # [S, 64]
    cos = np.cos(ang).T.astype(np.float32)          # [64, S]
    sin = np.sin(ang).T.astype(np.float32)
    cc1 = np.concatenate([cos, cos], 0)             # [128, S]
    ss1 = np.concatenate([-sin, sin], 0)            # [128, S]
    return cc1.astype(BF16), ss1.astype(BF16)


def _prep_inputs(x, W_qkv, W_o):
    x = np.asarray(x, dtype=np.float32)
    W_qkv = np.asarray(W_qkv, dtype=np.float32)
    W_o = np.asarray(W_o, dtype=np.float32)

    xTd = [
        np.ascontiguousarray(
            x[b].T.reshape(HT, 128, S).transpose(1, 0, 2)
        ).astype(BF16)                               # [128, HT, 2048]
        for b in range(B)
    ]

    cc, ss = _rope_tables()

    mask = np.zeros((128, 4, 1024), dtype=np.float32)
    ii = np.arange(128)[:, None]
    jj = np.arange(512)[None, :]
    for r in range(4):
        m = (jj >= ii + 128 * r)
        mask[:, r, 0:512] = m
        mask[:, r, 512:1024] = m
    mask = mask.astype(BF16)

    ones128 = np.ones((128, 128), dtype=np.float32).astype(BF16)
    ident = np.eye(128, dtype=np.float32).astype(BF16)

    # gathered feature-tile order: block hp*8+t holds feature tile
    # f128 = 4*(t//2) + 2*hp + t%2  (rank-major AllGather of head pairs)
    _DTT = [4 * (t // 2) + 2 * hp + t % 2 for hp in range(2) for t in range(8)]

    in_maps = []
    for c in range(8):
        b, g = c // 4, c % 4
        qr = W_qkv[512 * g: 512 * (g + 1)]           # rows of q heads 4g..4g+3
        qr = qr.reshape(NQ, HD, HIDDEN)[:, _PERM, :].reshape(512, HIDDEN)
        kr = W_qkv[HIDDEN + 128 * g: HIDDEN + 128 * (g + 1)][_PERM, :]
        vr = W_qkv[HIDDEN + 512 + 128 * g: HIDDEN + 512 + 128 * (g + 1)]
        wqkvT = np.ascontiguousarray(
            np.concatenate([qr, kr, vr], 0).T.reshape(HT, 128, 768).transpose(1, 0, 2)
        ).astype(BF16)                               # [128, HT, 768]
        woT = np.ascontiguousarray(
            W_o[256 * c: 256 * (c + 1)].T.reshape(HT, 128, 256)[_DTT].transpose(1, 0, 2)
        ).astype(BF16)                               # [128, HT(reordered), 256]
        in_maps.append({
            "xT": xTd[b], "wqkv": wqkvT, "wo": woT,
            "cc": cc, "ss": ss, "msk": mask, "ones128": ones128, "ident": ident,
        })
    return in_maps


def kernel(x, W_qkv, W_o):
    global _COMPILED
    if _COMPILED is None:
        _COMPILED = _build()
    nc = _COMPILED
    in_maps = _prep_inputs(x, W_qkv, W_o)
    res = run_bass_kernel_spmd(nc, in_maps, list(range(8)))
    out = np.empty((B, S, HIDDEN), dtype=np.float32)
    for c in range(8):
        oT = res.results[c]["outT"]                  # [256, 4096]
        out[:, :, 256 * c: 256 * (c + 1)] = oT.reshape(256, B, S).transpose(1, 2, 0)
    return out


# revision 21
# speedup vs baseline: 1.0818x; 1.0546x over previous
"""Trainium2 Bass kernel for nn_Attention_4183298146960 — v6 sharding.

GQA causal attention layer: B=2, S=2048, HIDDEN=2048, 16 q heads / 4 kv heads,
head_dim=128, RoPE (interleaved pairs), causal softmax, output projection.

Sharding v6 (8 cores, SPMD-uniform program, per-core inputs differ):
  core c owns batch c//4 and kv-head g=c%4 with its 4 q heads {4g..4g+3}.
  Eliminates the duplicated K/V projections of head-sharding (each (batch,
  kv-head) is computed exactly once), halves the x DMA per core, and the
  output-projection gather becomes an AllGather within each 4-core batch
  group (6 MB received per core instead of 14 MB).

Pipeline: per 512-token tile j (= qt chunk of my batch): QKV (q0..q3,k,v as
six N=512 column blocks of one packed weight), RoPE on q/k, PE-transpose of
v to token-major, then the causal attention chunk over k-tiles 0..4qt+3 in
two head-pair passes, its AllGather, the previous chunk's gather-readback
(emitted before the new AllGather so the framework's shared collective
counter waits only on the producing gather), and a deferred chunk's W_o.

Layouts as before: feature-major qT/kT for scores, token-major v for PV,
scores [k, q] with a head-pair side by side in one [128, 1024] PSUM tile;
unnormalized exp, ones-matmul denominator, late division.
"""

import numpy as np
import ml_dtypes

import concourse.bass as bass
import concourse.mybir as mybir
import concourse.tile as tile
from concourse import bacc
from concourse.bass_utils import run_bass_kernel_spmd

BF16 = ml_dtypes.bfloat16

HEADS = 16
KV_HEADS = 4
HIDDEN = 2048
HD = 128
S = 2048
B = 2
HT = HIDDEN // 128             # 16 hidden tiles
NQ = 4                         # local q heads
SCALE = 1.0 / float(np.sqrt(HD))
RG8 = [[0, 1, 2, 3, 4, 5, 6, 7]]

_COMPILED = None


def _build():
    dt = mybir.dt
    nc = bacc.Bacc("TRN2", target_bir_lowering=False, debug=False, num_devices=8)

    xT = nc.dram_tensor("xT", [128, HT, S], dt.bfloat16, kind="ExternalInput")
    wqkv = nc.dram_tensor("wqkv", [128, HT, 768], dt.bfloat16, kind="ExternalInput")
    wo = nc.dram_tensor("wo", [128, HT, 256], dt.bfloat16, kind="ExternalInput")
    cc = nc.dram_tensor("cc", [128, S], dt.bfloat16, kind="ExternalInput")
    ss = nc.dram_tensor("ss", [128, S], dt.bfloat16, kind="ExternalInput")
    msk = nc.dram_tensor("msk", [128, 4, 1024], dt.bfloat16, kind="ExternalInput")
    ones128 = nc.dram_tensor("ones128", [128, 128], dt.bfloat16, kind="ExternalInput")
    ident = nc.dram_tensor("ident", [128, 128], dt.bfloat16, kind="ExternalInput")
    outT = nc.dram_tensor("outT", [256, B * S], dt.float32, kind="ExternalOutput")

    mult = mybir.AluOpType.mult
    add = mybir.AluOpType.add
    Exp = mybir.ActivationFunctionType.Exp

    with tile.TileContext(nc) as tc:
        with (
            tc.tile_pool(name="const", bufs=1) as constp,
            tc.tile_pool(name="dram", bufs=1, space="DRAM") as dram,
            tc.tile_pool(name="xp", bufs=2) as xp,
            tc.tile_pool(name="rp", bufs=3) as rp,
            tc.tile_pool(name="probs", bufs=3) as probs,
            tc.tile_pool(name="smallp", bufs=2) as smallp,
            tc.tile_pool(name="ap", bufs=3) as apool,
            tc.tile_pool(name="wosb", bufs=4) as wosb,
            tc.tile_pool(name="outp", bufs=1) as outp,
            # PSUM: qk 2 banks + scores 2x2 banks + pv 2 banks = 8
            tc.tile_pool(name="qkps", bufs=2, space="PSUM") as qkps,
            tc.tile_pool(name="spool", bufs=2, space="PSUM") as spool,
            tc.tile_pool(name="pvp", bufs=1, space="PSUM") as pvp,
        ):
            qcat = constp.tile([128, NQ * S], dt.bfloat16)   # 4 local q heads
            kT = constp.tile([128, S], dt.bfloat16)
            vsb = constp.tile([128, S], dt.bfloat16)         # token-major v
            wqkv_sb = constp.tile([128, HT, 768], dt.bfloat16)
            wo_sb = constp.tile([128, HT, 256], dt.bfloat16)
            cc_sb = constp.tile([128, S], dt.bfloat16)
            ss_sb = constp.tile([128, S], dt.bfloat16)
            msk_sb = constp.tile([128, 4, 1024], dt.bfloat16)
            ones_sb = constp.tile([128, 128], dt.bfloat16)
            id_sb = constp.tile([128, 128], dt.bfloat16)

            def load_x(j, x_sb):
                for hq in range(4):
                    nc.sync.dma_start(
                        x_sb[:, hq * 4:(hq + 1) * 4, :],
                        xT[:, hq * 4:(hq + 1) * 4, j * 512:(j + 1) * 512],
                    )

            # interleave weight/x chunks so the first ht-tile MMs start ASAP
            x_tiles = {}
            x_tiles[0] = xp.tile([128, HT, 512], dt.bfloat16, name="x0", tag="x")
            for lo, hi in ((0, 2), (2, 4), (4, 8), (8, 12), (12, 16)):
                nc.sync.dma_start(wqkv_sb[:, lo:hi, :], wqkv[:, lo:hi, :])
                nc.sync.dma_start(
                    x_tiles[0][:, lo:hi, :], xT[:, lo:hi, 0:512],
                )

            nc.sync.dma_start(cc_sb[:, 0:512], cc[:, 0:512])
            nc.sync.dma_start(ss_sb[:, 0:512], ss[:, 0:512])
            nc.sync.dma_start(id_sb[:], ident[:])
            nc.sync.dma_start(cc_sb[:, 512:S], cc[:, 512:S])
            nc.sync.dma_start(ss_sb[:, 512:S], ss[:, 512:S])
            nc.sync.dma_start(msk_sb[:], msk[:])
            nc.sync.dma_start(ones_sb[:], ones128[:])
            nc.sync.dma_start(wo_sb[:], wo[:])

            def emit_qkv(j, x_sb):
                """QKV + RoPE + v-transpose for t-tile j (512 tokens)."""
                tsl = bass.ts(j, 512)
                for ft in range(6):
                    ps = qkps.tile([128, 512], dt.float32, tag="qk")
                    for ht in range(HT):
                        nc.tensor.matmul(
                            ps[:],
                            lhsT=wqkv_sb[:, ht, ft * 128:(ft + 1) * 128],
                            rhs=x_sb[:, ht, :],
                            start=(ht == 0),
                            stop=(ht == HT - 1),
                        )
                    if ft < 5:
                        # q0..q3, k: RoPE
                        sbq = rp.tile([128, 512], dt.bfloat16, name="sbq")
                        nc.scalar.copy(sbq[:], ps[:])
                        tmp = rp.tile([128, 512], dt.bfloat16, name="tmp")
                        nc.gpsimd.dma_start(tmp[0:64, :], sbq[64:128, :])
                        nc.gpsimd.dma_start(tmp[64:128, :], sbq[0:64, :])
                        qcc = rp.tile([128, 512], dt.bfloat16, name="qcc")
                        nc.vector.tensor_tensor(qcc[:], sbq[:], cc_sb[:, tsl], mult)
                        qss = rp.tile([128, 512], dt.bfloat16, name="qss")
                        nc.vector.tensor_tensor(qss[:], tmp[:], ss_sb[:, tsl], mult)
                        if ft < 4:
                            dst = qcat[:, ft * S + j * 512: ft * S + (j + 1) * 512]
                        else:
                            dst = kT[:, tsl]
                        nc.vector.tensor_tensor(dst, qcc[:], qss[:], add)
                    else:
                        # v: feature-major -> PE transpose -> token-major
                        vf = rp.tile([128, 512], dt.bfloat16, name="vf")
                        nc.scalar.copy(vf[:], ps[:])
                        pt = qkps.tile(
                            [128, 512], dt.bfloat16, tag="qk", name="pt",
                            padded_shape=[128, 1024],
                        )
                        for st in range(4):
                            nc.tensor.transpose(
                                pt[:, st * 128:(st + 1) * 128],
                                vf[:, st * 128:(st + 1) * 128],
                                id_sb[:],
                            )
                        nc.vector.tensor_copy(vsb[:, tsl], pt[:])

            def emit_attn_pass(qt, hp):
                """Causal attention for 512 q tokens, one head pair; ends with
                the pair's 8-rank AllGather (mesh algorithm; the gathered
                tensor holds both batch groups — W_o reads are batch-split by
                output token range, so the program stays SPMD-uniform)."""
                kts = 4 * qt + 4
                attn_chunk = dram.tile(
                    [256, 512], dt.bfloat16, name=f"attnc{qt}_{hp}", tag=f"ac{qt}{hp}"
                )
                pv = pvp.tile([128, 1024], dt.float32, tag="pv")
                acc = smallp.tile([128, 1024], dt.bfloat16, name="acc", tag="acc")
                for kt in range(kts):
                    r = kt - 4 * qt
                    ps_s = spool.tile([128, 1024], dt.float32, tag="sc")
                    for hl in range(2):
                        h = 2 * hp + hl
                        nc.tensor.matmul(
                            ps_s[:, hl * 512:(hl + 1) * 512],
                            lhsT=kT[:, kt * 128:(kt + 1) * 128],
                            rhs=qcat[:, h * S + qt * 512: h * S + (qt + 1) * 512],
                            start=True,
                            stop=True,
                        )
                    prob = probs.tile([128, 1024], dt.bfloat16, tag="prob")
                    nc.scalar.activation(prob[:], ps_s[:], Exp, scale=SCALE)
                    if r >= 0:
                        nc.vector.tensor_tensor(
                            prob[:], prob[:], msk_sb[:, r, :], mult
                        )
                    for hl in range(2):
                        nc.tensor.matmul(
                            pv[:, hl * 512:(hl + 1) * 512],
                            lhsT=vsb[:, kt * 128:(kt + 1) * 128],
                            rhs=prob[:, hl * 512:(hl + 1) * 512],
                            start=(kt == 0), stop=(kt == kts - 1),
                        )
                    if kt == 0:
                        nc.vector.tensor_copy(acc[:], prob[:])
                    else:
                        nc.vector.tensor_add(acc[:], acc[:], prob[:])
                ps_den = spool.tile([128, 1024], dt.float32, tag="sc", name="ps_den")
                for hl in range(2):
                    nc.tensor.matmul(
                        ps_den[:, hl * 512:(hl + 1) * 512],
                        lhsT=ones_sb[:], rhs=acc[:, hl * 512:(hl + 1) * 512],
                        start=True, stop=True,
                    )
                den_sb = smallp.tile([128, 1024], dt.float32, name="den", tag="den")
                nc.vector.reciprocal_approx_fast(den_sb[:], ps_den[:])
                for hl in range(2):
                    attn_sb = apool.tile([128, 512], dt.bfloat16, tag="asb")
                    nc.vector.tensor_tensor(
                        attn_sb[:], pv[:, hl * 512:(hl + 1) * 512],
                        den_sb[:, hl * 512:(hl + 1) * 512], mult,
                    )
                    nc.sync.dma_start(
                        attn_chunk[hl * 128:(hl + 1) * 128, :], attn_sb[:]
                    )
                ag_out = dram.tile(
                    [HT, 128, 512], dt.bfloat16,
                    addr_space="Shared", name=f"agout{qt}_{hp}", tag=f"ag{qt}{hp}",
                )
                nc.gpsimd.collective_compute(
                    "AllGather", mybir.AluOpType.bypass, replica_groups=RG8,
                    ins=[attn_chunk.opt()], outs=[ag_out.opt()],
                )
                return ag_out

            def load_asb(ag_out):
                # NB: must be emitted BEFORE the next collective so the
                # framework's shared collective-completion counter makes this
                # wait only on the producing AllGather, not a later one.
                asb = wosb.tile([128, HT, 512], dt.bfloat16, tag="asb")
                for g in range(4):
                    nc.sync.dma_start(
                        asb[:, g * 4:(g + 1) * 4, :],
                        ag_out[g * 4:(g + 1) * 4, :, :].transpose([1, 0, 2]),
                    )
                return asb

            def emit_wo(qt, asb01):
                """W_o for one chunk: my 256 output columns for both batches'
                512 tokens, contracting over both head-pair gathers.
                Gathered rows: blocks 0-7 batch0, 8-15 batch1 (rank-major)."""
                for b in range(B):
                    for ct in range(2):
                        ps_o = qkps.tile([128, 512], dt.float32, tag="qk", name="ps_o")
                        for hp in range(2):
                            for t in range(8):
                                nc.tensor.matmul(
                                    ps_o[:],
                                    lhsT=wo_sb[:, hp * 8 + t, ct * 128:(ct + 1) * 128],
                                    rhs=asb01[hp][:, b * 8 + t, :],
                                    start=(hp == 0 and t == 0),
                                    stop=(hp == 1 and t == 7),
                                )
                        o_sb = outp.tile([128, 512], dt.float32, tag="osb")
                        nc.scalar.copy(o_sb[:], ps_o[:])
                        nc.scalar.dma_start(
                            outT[ct * 128:(ct + 1) * 128,
                                 b * S + qt * 512:b * S + (qt + 1) * 512],
                            o_sb[:],
                        )

            pending_ag = []    # gathered passes, asb not yet loaded
            loaded = []        # (qt, [asb_hp0, asb_hp1]) ready for W_o
            for j in range(4):
                if j + 1 < 4:
                    x_tiles[j + 1] = xp.tile(
                        [128, HT, 512], dt.bfloat16, name=f"x{j + 1}", tag="x"
                    )
                    load_x(j + 1, x_tiles[j + 1])
                emit_qkv(j, x_tiles[j])
                # read back the previous chunk's two gathers before emitting
                # this iteration's AllGathers
                if pending_ag:
                    qq, ag0, ag1 = pending_ag.pop(0)
                    loaded.append((qq, [load_asb(ag0), load_asb(ag1)]))
                ag0 = emit_attn_pass(j, 0)
                ag1 = emit_attn_pass(j, 1)
                if len(loaded) > 1:
                    qq, asb01 = loaded.pop(0)
                    emit_wo(qq, asb01)
                pending_ag.append((j, ag0, ag1))
            # tail: chunk 2 (already loaded) covers the final gather's
            # latency, then chunk 3
            qq, asb01 = loaded.pop(0)
            emit_wo(qq, asb01)
            qq, ag0, ag1 = pending_ag.pop(0)
            asb01 = [load_asb(ag0), load_asb(ag1)]
            emit_wo(qq, asb01)
    nc.compile()
    return nc


# host-side input prep ------------------------------------------------------

_PERM = np.concatenate([np.arange(0, HD, 2), np.arange(1, HD, 2)])


def _rope_tables():
    freq = 1.0 / (10000.0 ** (np.arange(0, HD, 2, dtype=np.float64) / HD))
    pos = np.arange(S, dtype=np.float64)
    ang = np.outer(pos, freq)                       # [S, 64]
    cos = np.cos(ang).T.astype(np.float32)          # [64, S]
    sin = np.sin(ang).T.astype(np.float32)
    cc1 = np.concatenate([cos, cos], 0)             # [128, S]
    ss1 = np.concatenate([-sin, sin], 0)            # [128, S]
    return cc1.astype(BF16), ss1.astype(BF16)


def _prep_inputs(x, W_qkv, W_o):
    x = np.asarray(x, dtype=np.float32)
    W_qkv = np.asarray(W_qkv, dtype=np.float32)
    W_o = np.asarray(W_o, dtype=np.float32)

    xTd = [
        np.ascontiguousarray(
            x[b].T.reshape(HT, 128, S).transpose(1, 0, 2)
        ).astype(BF16)                               # [128, HT, 2048]
        for b in range(B)
    ]

    cc, ss = _rope_tables()

    mask = np.zeros((128, 4, 1024), dtype=np.float32)
    ii = np.arange(128)[:, None]
    jj = np.arange(512)[None, :]
    for r in range(4):
        m = (jj >= ii + 128 * r)
        mask[:, r, 0:512] = m
        mask[:, r, 512:1024] = m
    mask = mask.astype(BF16)

    ones128 = np.ones((128, 128), dtype=np.float32).astype(BF16)
    ident = np.eye(128, dtype=np.float32).astype(BF16)

    # gathered feature-tile order: block hp*8+t holds feature tile
    # f128 = 4*(t//2) + 2*hp + t%2  (rank-major AllGather of head pairs)
    _DTT = [4 * (t // 2) + 2 * hp + t % 2 for hp in range(2) for t in range(8)]

    in_maps = []
    for c in range(8):
        b, g = c // 4, c % 4
        qr = W_qkv[512 * g: 512 * (g + 1)]           # rows of q heads 4g..4g+3
        qr = qr.reshape(NQ, HD, HIDDEN)[:, _PERM, :].reshape(512, HIDDEN)
        kr = W_qkv[HIDDEN + 128 * g: HIDDEN + 128 * (g + 1)][_PERM, :]
        vr = W_qkv[HIDDEN + 512 + 128 * g: HIDDEN + 512 + 128 * (g + 1)]
        wqkvT = np.ascontiguousarray(
            np.concatenate([qr, kr, vr], 0).T.reshape(HT, 128, 768).transpose(1, 0, 2)
        ).astype(BF16)                               # [128, HT, 768]
        woT = np.ascontiguousarray(
            W_o[256 * c: 256 * (c + 1)].T.reshape(HT, 128, 256)[_DTT].transpose(1, 0, 2)
        ).astype(BF16)                               # [128, HT(reordered), 256]
        in_maps.append({
            "xT": xTd[b], "wqkv": wqkvT, "wo": woT,
            "cc": cc, "ss": ss, "msk": mask, "ones128": ones128, "ident": ident,
        })
    return in_maps


def kernel(x, W_qkv, W_o):
    global _COMPILED
    if _COMPILED is None:
        _COMPILED = _build()
    nc = _COMPILED
    in_maps = _prep_inputs(x, W_qkv, W_o)
    res = run_bass_kernel_spmd(nc, in_maps, list(range(8)))
    out = np.empty((B, S, HIDDEN), dtype=np.float32)
    for c in range(8):
        oT = res.results[c]["outT"]                  # [256, 4096]
        out[:, :, 256 * c: 256 * (c + 1)] = oT.reshape(256, B, S).transpose(1, 2, 0)
    return out
